# revision 1
# baseline (speedup 1.0000x reference)
"""Trainium2 Bass kernel for a pre-LN transformer block (B=2,S=2048,H=1024,NH=16,FFN=4096).

Sharding: 8 cores, 512 tokens/core (4 cores per batch element). K/V are
exchanged within each batch group via one 4-rank AllGather (bf16). All matmuls
run in bf16 on the PE array with fp32 PSUM accumulation; LayerNorm statistics,
residuals and the final output stay fp32.

Self-contained: hardcodes shapes; builds the Bass program once and runs it via
run_bass_kernel_spmd on cores 0-7.
"""

import sys

for _p in ("/root/.axon_site/_ro/trn_rl_repo", "/opt/trn_rl_repo"):
    if _p not in sys.path:
        sys.path.append(_p)

import numpy as np
import ml_dtypes

# If BASS_TRACE is set but the axon NTFF hook module is missing, the trace
# path would crash on import; pre-register a no-op hook shim so tracing
# degrades gracefully instead.
try:
    import antenv.axon_hooks  # noqa: F401
except ImportError:
    import types as _types
    _m = _types.ModuleType("antenv.axon_hooks")
    _m._hook = None
    _m.get_axon_ntff_profile_hook = lambda: _m._hook
    _m.set_axon_ntff_profile_hook = lambda h: setattr(_m, "_hook", h)
    sys.modules["antenv.axon_hooks"] = _m

import bass_rust
import concourse.bass as bass
import concourse.mybir as mybir
import concourse.tile as tile
from concourse.bass_utils import run_bass_kernel_spmd

BF16 = mybir.dt.bfloat16
F32 = mybir.dt.float32
AF = mybir.ActivationFunctionType
NPBF16 = np.dtype(ml_dtypes.bfloat16)

B, S, H, NH, DH, FFN = 2, 2048, 1024, 16, 64, 4096
NC = 8                      # cores
T = 512                     # tokens per core
NT = T // 128               # token tiles per core (4)
GROUPS = [[0, 1, 2, 3], [4, 5, 6, 7]]
G = 4                       # cores per batch group
SKEYS = S                   # keys per batch (2048)
NKT = SKEYS // 128          # key tiles (16)
NHP = NH // 2               # head pairs (8)
EPS = 1e-3
VW = DH + 1                 # 65: V columns + ones column per head
KV_CHUNK = T * H            # bf16 elems per (kT | v) contribution: 524288
# key tiles in half-A (first V AllGather) then half-B order
KT_HALF_A = [g * 4 + j for g in range(4) for j in (0, 1)]
KT_HALF_B = [g * 4 + 2 + j for g in range(4) for j in (0, 1)]
KT_PAIRS = [tuple(KT_HALF_A[i:i + 2]) for i in range(0, 8, 2)] + \
           [tuple(KT_HALF_B[i:i + 2]) for i in range(0, 8, 2)]

# ---------------------------------------------------------------------------
# Workaround: this walrus build rejects >1 inline sync-wait per instruction.
# After Tile scheduling, move excess waits onto single-wait NoOp carriers
# inserted immediately before the over-limit instruction (same engine, same
# block, so per-engine program order and wait semantics are preserved).
# ---------------------------------------------------------------------------
def _split_multiwait(nc, limit=1):
    n_new = 0
    for f in nc.m.functions:
        for blk in f.blocks:
            insts = blk.instructions
            out = []
            for ins in insts:
                si = getattr(ins, "sync_info", None)
                waits = list(si.on_wait) if si is not None else []
                if len(waits) > limit:
                    for i, w in enumerate(waits[:-limit]):
                        nop = mybir.InstNoOp(
                            name=f"{ins.name}_w{i}",
                            sync_info=mybir.SyncInfo(on_wait=[w], on_update=[]),
                            bass_nofuse=True,
                            engine=ins.engine,
                        )
                        out.append(nop)
                        n_new += 1
                    ins.sync_info = mybir.SyncInfo(
                        on_wait=waits[-limit:], on_update=list(si.on_update)
                    )
                out.append(ins)
            if len(out) != len(insts):
                blk.instructions = out
    return n_new


def _emit(tc, nc, io):
    """Emit the per-core program. io: dict of DRAM APs."""
    from contextlib import ExitStack

    x_d = io["x"]
    out_d = io["out"]

    # ---- long-lived pools. Tile pools must close in LIFO order: keep all
    # persistent tensors in one outer pool (held to the end); each phase's
    # scratch lives in phase-local pools that close before the next opens. ----
    s_outer = ExitStack()

    constp = s_outer.enter_context(tc.tile_pool(name="constp", bufs=1))
    dramp = s_outer.enter_context(tc.tile_pool(name="dramp", bufs=1, space="DRAM"))

    # constants / biases
    ident = constp.tile([128, 128], BF16)
    nc.sync.dma_start(ident[:], io["ident"][:])
    ones_row = constp.tile([1, 128], BF16)
    nc.sync.dma_start(ones_row[:], io["ones_row"][:])
    bq = constp.tile([128, 8], F32); nc.sync.dma_start(bq[:], io["bq"][:])
    bk = constp.tile([128, 8], F32); nc.sync.dma_start(bk[:], io["bk"][:])
    bi = constp.tile([128, 32], F32); nc.sync.dma_start(bi[:], io["bi"][:])
    bv_bf = constp.tile([1, H], BF16); nc.sync.dma_start(bv_bf[:], io["bv_bf"][:])
    bproj_bf = constp.tile([1, H], BF16); nc.sync.dma_start(bproj_bf[:], io["bproj_bf"][:])
    bo_bf = constp.tile([1, H], BF16); nc.sync.dma_start(bo_bf[:], io["bo_bf"][:])
    eps_t = constp.tile([128, 1], F32); nc.gpsimd.memset(eps_t[:], float(EPS))

    # collective buffers (AllGather concatenates along dim 0); both K and V
    # are gathered in two halves so attention can start on the first half
    cc_k_in_a = dramp.tile([512, T], BF16)
    cc_k_in_b = dramp.tile([512, T], BF16)
    cc_k_out_a = dramp.tile([G * 512, T], BF16)
    cc_k_out_b = dramp.tile([G * 512, T], BF16)
    cc_v_in_a = dramp.tile([256, H], BF16)
    cc_v_in_b = dramp.tile([256, H], BF16)
    cc_v_out_a = dramp.tile([G * 256, H], BF16)
    cc_v_out_b = dramp.tile([G * 256, H], BF16)

    # persistent activations (one outer pool, held until the end)
    persp = s_outer.enter_context(tc.tile_pool(name="persp", bufs=1))
    x2_all = persp.tile([128, NT * H], F32, name="x2_all")
    h3T_all = persp.tile([128, 32 * T], BF16, name="h3T_all")
    h2T_all = persp.tile([128, 8 * T], BF16, name="h2T_all")
    ctxT_all = persp.tile([128, 8 * T], BF16, name="ctxT_all")
    wproj_sb = persp.tile([128, 8 * H], BF16, name="wproj_sb")
    x_all = persp.tile([128, NT * H], F32, name="x_all")
    qT_all = persp.tile([128, 8 * T], BF16, name="qT_all")

    def layer_norm_to(pool, h_out_slice, x_slice):
        """x_slice [128,H] f32 -> h_out_slice [128,H] bf16 standardized."""
        sq = pool.tile([128, H], F32, tag="ln_sq")
        nsum = pool.tile([128, 1], F32, tag="ln_nsum")
        s2 = pool.tile([128, 1], F32, tag="ln_s2")
        var = pool.tile([128, 1], F32, tag="ln_var")
        std = pool.tile([128, 1], F32, tag="ln_std")
        rs = pool.tile([128, 1], F32, tag="ln_rs")
        nmu = pool.tile([128, 1], F32, tag="ln_nmu")
        nmurs = pool.tile([128, 1], F32, tag="ln_nmurs")
        nc.vector.reduce_sum(nsum[:], x_slice, axis=mybir.AxisListType.X, negate=True)
        nc.vector.tensor_mul(sq[:], x_slice, x_slice)
        nc.vector.reduce_sum(s2[:], sq[:], axis=mybir.AxisListType.X)
        nc.vector.tensor_scalar_mul(nmu[:], nsum[:], 1.0 / H)      # -mean
        nc.vector.tensor_scalar_mul(s2[:], s2[:], 1.0 / H)         # E[x^2]
        nc.vector.tensor_mul(var[:], nmu[:], nmu[:])               # mean^2
        nc.vector.tensor_sub(var[:], s2[:], var[:])                # var
        nc.scalar.activation(std[:], var[:], AF.Sqrt, bias=eps_t[:])
        nc.vector.reciprocal(rs[:], std[:])
        nc.vector.tensor_mul(nmurs[:], nmu[:], rs[:])              # -mean*rs
        nc.scalar.activation(h_out_slice, x_slice, AF.Identity, bias=nmurs[:], scale=rs[:])

    def transpose_128(dst_slice, src_slice, tps, cpool):
        """PE-transpose src [128,128] bf16 -> dst [128,128] bf16."""
        ps = tps.tile([128, 128], BF16, tag="tp")
        nc.tensor.transpose(ps[:], src_slice, ident[:])
        nc.vector.tensor_copy(dst_slice, ps[:])

    # =====================================================================
    # Phase A: load x, LN1, h1^T, q^T/k^T (feature-major), v (token-major)
    # =====================================================================
    sA = ExitStack()
    wq_p = sA.enter_context(tc.tile_pool(name="wq_p", bufs=2))
    lnp = sA.enter_context(tc.tile_pool(name="lnp", bufs=2))
    h1p = sA.enter_context(tc.tile_pool(name="h1p", bufs=1))
    h1Tp = sA.enter_context(tc.tile_pool(name="h1Tp", bufs=1))
    tpsA = sA.enter_context(tc.tile_pool(name="tpsA", bufs=2, space="PSUM"))
    mmpsA = sA.enter_context(tc.tile_pool(name="mmpsA", bufs=2, space="PSUM"))
    stgA = sA.enter_context(tc.tile_pool(name="stgA", bufs=4))

    h1_all = h1p.tile([128, NT * H], BF16)
    h1T_all = h1Tp.tile([128, 8 * T], BF16)

    # x + wk load first so the K^T -> AllGather chain is not queued behind
    # the other weight DMAs
    for t in range(NT):
        nc.sync.dma_start(x_all[:, t * H:(t + 1) * H], x_d[t * 128:(t + 1) * 128, :])
    wk_sb = wq_p.tile([128, 8 * H], BF16, tag="w3", name="wk_sb")
    for fb in range(8):
        nc.sync.dma_start(wk_sb[:, fb * H:(fb + 1) * H], io["wk"][fb * 128:(fb + 1) * 128, :])

    for t in range(NT):
        layer_norm_to(lnp, h1_all[:, t * H:(t + 1) * H], x_all[:, t * H:(t + 1) * H])
        for fb in range(8):
            transpose_128(
                h1T_all[:, fb * T + t * 128: fb * T + (t + 1) * 128],
                h1_all[:, t * H + fb * 128: t * H + (fb + 1) * 128],
                tpsA, stgA,
            )

    # k^T feature-major: [128 feats, T] per col-tile; AllGather ASAP
    for ct in range(8):
        ps = mmpsA.tile([128, T], F32, tag="mm_qk")
        for fb in range(8):
            nc.tensor.matmul(
                ps[:],
                wk_sb[:, fb * H + ct * 128: fb * H + (ct + 1) * 128],
                h1T_all[:, fb * T:(fb + 1) * T],
                start=(fb == 0), stop=(fb == 7),
            )
        ktmp = stgA.tile([128, T], BF16, tag="ktmp")
        nc.scalar.activation(ktmp[:], ps[:], AF.Identity, bias=bk[:, ct:ct + 1])
        dst = cc_k_in_a if ct < 4 else cc_k_in_b
        nc.sync.dma_start(dst[(ct % 4) * 128:(ct % 4 + 1) * 128, :], ktmp[:])

    nc.gpsimd.collective_compute(
        "AllGather", mybir.AluOpType.bypass, replica_groups=GROUPS,
        ins=[cc_k_in_a.opt()], outs=[cc_k_out_a.opt()],
    )

    # q^T feature-major (runs while the K AllGather is in flight)
    wq_sb = wq_p.tile([128, 8 * H], BF16, tag="w3", name="wq_sb")
    for fb in range(8):
        nc.sync.dma_start(wq_sb[:, fb * H:(fb + 1) * H], io["wq"][fb * 128:(fb + 1) * 128, :])
    for ct in range(8):
        ps = mmpsA.tile([128, T], F32, tag="mm_qk")
        for fb in range(8):
            nc.tensor.matmul(
                ps[:],
                wq_sb[:, fb * H + ct * 128: fb * H + (ct + 1) * 128],
                h1T_all[:, fb * T:(fb + 1) * T],
                start=(fb == 0), stop=(fb == 7),
            )
        nc.scalar.activation(
            qT_all[:, ct * T:(ct + 1) * T], ps[:], AF.Identity,
            bias=bq[:, ct:ct + 1])

    # v token-major: [128 tok, H] (wv reuses the wk slot once kT is done)
    wv_sb = wq_p.tile([128, 8 * H], BF16, tag="w3", name="wv_sb")
    for fb in range(8):
        nc.sync.dma_start(wv_sb[:, fb * H:(fb + 1) * H], io["wv"][fb * 128:(fb + 1) * 128, :])
    for t in range(NT):
        for cc in range(2):
            ps = mmpsA.tile([128, 512], F32, tag="mm_v")
            for fb in range(8):
                nc.tensor.matmul(
                    ps[:],
                    h1T_all[:, fb * T + t * 128: fb * T + (t + 1) * 128],
                    wv_sb[:, fb * H + cc * 512: fb * H + (cc + 1) * 512],
                    start=(fb == 0), stop=False,
                )
            nc.tensor.matmul(ps[:], ones_row[:], bv_bf[:, cc * 512:(cc + 1) * 512],
                             start=False, stop=True)
            vtmp = stgA.tile([128, 512], BF16, tag="vtmp")
            nc.vector.tensor_copy(vtmp[:], ps[:])
            dst = cc_v_in_a if t < 2 else cc_v_in_b
            nc.sync.dma_start(dst[(t % 2) * 128:(t % 2 + 1) * 128, cc * 512:(cc + 1) * 512], vtmp[:])

    nc.gpsimd.collective_compute(
        "AllGather", mybir.AluOpType.bypass, replica_groups=GROUPS,
        ins=[cc_v_in_a.opt()], outs=[cc_v_out_a.opt()],
    )
    nc.gpsimd.collective_compute(
        "AllGather", mybir.AluOpType.bypass, replica_groups=GROUPS,
        ins=[cc_v_in_b.opt()], outs=[cc_v_out_b.opt()],
    )
    nc.gpsimd.collective_compute(
        "AllGather", mybir.AluOpType.bypass, replica_groups=GROUPS,
        ins=[cc_k_in_b.opt()], outs=[cc_k_out_b.opt()],
    )

    sA.close()

    # prefetch proj weights during attention
    for hp in range(8):
        nc.sync.dma_start(wproj_sb[:, hp * H:(hp + 1) * H],
                          io["wproj"][hp * 128:(hp + 1) * 128, :])

    # =====================================================================
    # Phase B: attention. scores^T per key-tile (row-packed head pairs),
    # exp on ACT, ctx^T via V'=[V|ones] (M=65), normalize with 1/sumexp.
    # =====================================================================
    sB = ExitStack()
    vstg = sB.enter_context(tc.tile_pool(name="vstg", bufs=4))
    kpool = sB.enter_context(tc.tile_pool(name="kpool", bufs=3))
    spool = sB.enter_context(tc.tile_pool(name="spool", bufs=2, space="PSUM"))
    cpool = sB.enter_context(tc.tile_pool(name="cpool", bufs=2, space="PSUM"))
    ppool = sB.enter_context(tc.tile_pool(name="ppool", bufs=8))
    rpool = sB.enter_context(tc.tile_pool(name="rpool", bufs=2))
    vsb = sB.enter_context(tc.tile_pool(name="vsb_p", bufs=1)).tile([128, NKT * NH * VW], BF16, name="vsb")

    def load_kt(hp):
        kt_hp = kpool.tile([128, SKEYS], BF16, tag="kt_hp", name="kt_hp")
        cko, hpo = (cc_k_out_a, hp) if hp < 4 else (cc_k_out_b, hp - 4)
        for g in range(G):
            nc.sync.dma_start(kt_hp[:, g * T:(g + 1) * T],
                              cko[g * 512 + hpo * 128: g * 512 + (hpo + 1) * 128, :])
        return kt_hp

    # ones columns for all key tiles up front (DVE; no data deps)
    for kt in range(NKT):
        blk = vsb[:, kt * NH * VW:(kt + 1) * NH * VW]
        nc.vector.memset(blk.rearrange("p (h x) -> p h x", x=VW)[:, :, DH:VW], 1.0)

    def load_v_half(half_kts, cc_v_out):
        # V from AllGather output, interleaving a ones column per head
        for kt in half_kts:
            g, j = kt // 4, (kt % 4) % 2
            vplain = vstg.tile([128, H], BF16, tag="vplain", name="vplain")
            nc.sync.dma_start(vplain[:], cc_v_out[g * 256 + j * 128: g * 256 + (j + 1) * 128, :])
            blk = vsb[:, kt * NH * VW:(kt + 1) * NH * VW]
            dst = blk.rearrange("p (h x) -> p h x", x=VW)
            nc.vector.tensor_copy(dst[:, :, 0:DH], vplain[:].rearrange("p (h d) -> p h d", d=DH))

    # SP-queue emission order: kt prefetches for the first head pairs, then
    # the V halves (gated on the later AllGathers), then the rest
    kt_tiles = [None] * NHP
    for hp in range(4):
        kt_tiles[hp] = load_kt(hp)
    load_v_half(KT_HALF_A, cc_v_out_a)
    load_v_half(KT_HALF_B, cc_v_out_b)
    for hp in range(4, NHP):
        kt_tiles[hp] = load_kt(hp)

    for hp in range(NHP):
        kt_hp = kt_tiles[hp]
        cps0 = cpool.tile([128, T], F32, tag="ctx0")
        cps1 = cpool.tile([128, T], F32, tag="ctx1")

        def emit_ctx(kt, pb, first, last):
            for h, cps in enumerate((cps0, cps1)):
                head = hp * 2 + h
                nc.tensor.matmul(
                    cps[0:VW, :],
                    vsb[:, kt * NH * VW + head * VW: kt * NH * VW + (head + 1) * VW],
                    pb[:, h * 512:(h + 1) * 512],
                    start=first, stop=last,
                )

        # software-pipelined: scores(kt) | ctx(kt-1) | exp(kt).  Both heads'
        # scores share one [128,1024] psum so a single exp releases the slot
        # and the row-tiled pair stays adjacent (concurrent on the PE).
        KT_ORDER = KT_HALF_A + KT_HALF_B
        prev = None
        for pi, kt in enumerate(KT_ORDER):
            ps = spool.tile([128, 1024], F32, tag="ps", name="ps")
            nc.tensor.matmul(
                ps[:, 0:512],
                kt_hp[0:64, kt * 128:(kt + 1) * 128],
                qT_all[0:64, hp * T:(hp + 1) * T],
                start=True, stop=True, tile_position=(0, 0),
            )
            nc.tensor.matmul(
                ps[:, 512:1024],
                kt_hp[64:128, kt * 128:(kt + 1) * 128],
                qT_all[64:128, hp * T:(hp + 1) * T],
                start=True, stop=True, tile_position=(64, 0),
            )
            if prev is not None:
                emit_ctx(prev[0], prev[1], prev[2], False)
            pb = ppool.tile([128, 1024], BF16, tag="pb", name="pb")
            nc.scalar.activation(pb[:], ps[:], AF.Exp)
            prev = (kt, pb, pi == 0)
        emit_ctx(prev[0], prev[1], prev[2], True)
        rc0 = rpool.tile([1, T], F32, tag="rc0")
        rc1 = rpool.tile([1, T], F32, tag="rc1")
        nc.vector.reciprocal(rc0[:], cps0[DH:VW, :])
        nc.vector.reciprocal(rc1[:], cps1[DH:VW, :])
        rcb0 = rpool.tile([1, T], BF16, tag="rcb0")
        rcb1 = rpool.tile([1, T], BF16, tag="rcb1")
        nc.vector.tensor_copy(rcb0[:], rc0[:])
        nc.vector.tensor_copy(rcb1[:], rc1[:])
        # broadcast 1/sumexp across 64 partitions per head via K=1 matmuls
        bb = spool.tile([128, T], F32, tag="ps", name="bb")
        nc.tensor.matmul(bb[0:64, :], ones_row[:, 0:64], rcb0[:],
                         start=True, stop=True, tile_position=(0, 0))
        nc.tensor.matmul(bb[64:128, :], ones_row[:, 0:64], rcb1[:],
                         start=True, stop=True, tile_position=(0, 64))
        rb = rpool.tile([128, T], F32, tag="rb")
        nc.vector.tensor_copy(rb[:], bb[:])
        nc.vector.tensor_mul(ctxT_all[0:64, hp * T:(hp + 1) * T], cps0[0:DH, :], rb[0:64, :])
        nc.vector.tensor_mul(ctxT_all[64:128, hp * T:(hp + 1) * T], cps1[0:DH, :], rb[64:128, :])

    sB.close()

    # =====================================================================
    # Phase C: proj (token-major) + residual -> x2, LN2 -> h2^T
    # =====================================================================
    sC = ExitStack()
    lnp2 = sC.enter_context(tc.tile_pool(name="lnp2", bufs=2))
    h2p = sC.enter_context(tc.tile_pool(name="h2p", bufs=1))
    tpsC = sC.enter_context(tc.tile_pool(name="tpsC", bufs=2, space="PSUM"))
    mmpsC = sC.enter_context(tc.tile_pool(name="mmpsC", bufs=2, space="PSUM"))
    stgC = sC.enter_context(tc.tile_pool(name="stgC", bufs=4))

    h2_all = h2p.tile([128, NT * H], BF16)

    for t in range(NT):
        for cc in range(2):
            ps = mmpsC.tile([128, 512], F32, tag="pj")
            for hp in range(8):
                nc.tensor.matmul(
                    ps[:],
                    ctxT_all[:, hp * T + t * 128: hp * T + (t + 1) * 128],
                    wproj_sb[:, hp * H + cc * 512: hp * H + (cc + 1) * 512],
                    start=(hp == 0), stop=False,
                )
            nc.tensor.matmul(ps[:], ones_row[:], bproj_bf[:, cc * 512:(cc + 1) * 512],
                             start=False, stop=True)
            nc.vector.tensor_add(
                x2_all[:, t * H + cc * 512: t * H + (cc + 1) * 512],
                ps[:], x_all[:, t * H + cc * 512: t * H + (cc + 1) * 512])
        layer_norm_to(lnp2, h2_all[:, t * H:(t + 1) * H], x2_all[:, t * H:(t + 1) * H])
        for fb in range(8):
            transpose_128(
                h2T_all[:, fb * T + t * 128: fb * T + (t + 1) * 128],
                h2_all[:, t * H + fb * 128: t * H + (fb + 1) * 128],
                tpsC, stgC,
            )

    sC.close()

    # =====================================================================
    # Phase D+E fused: per g-tile: wi matmuls + gelu -> h3T[g], then wo
    # matmuls for output columns 0:512 accumulate into 4 persistent psums.
    # Second pass re-reads h3T for output columns 512:1024.
    # =====================================================================
    sD = ExitStack()
    wip = sD.enter_context(tc.tile_pool(name="wip", bufs=6))
    wop = sD.enter_context(tc.tile_pool(name="wop", bufs=6))
    mmpsD = sD.enter_context(tc.tile_pool(name="mmpsD", bufs=4, space="PSUM"))
    wops = sD.enter_context(tc.tile_pool(name="wops", bufs=1, space="PSUM"))
    outp = sD.enter_context(tc.tile_pool(name="outp", bufs=2))

    NG = FFN // 128  # 32
    psE = [wops.tile([128, 512], F32, tag=f"wo_ps{t}", name=f"wo_ps{t}") for t in range(NT)]
    for g in range(NG):
        wi_g = wip.tile([128, 8, 128], BF16, tag="wi_g", name="wi_g")
        src = io["wi"][g:g + 1, :, :, :].rearrange("o p f c -> (o p) f c")
        nc.sync.dma_start(wi_g[:], src)
        ps = mmpsD.tile([128, T], F32, tag="wi_ps", name="wi_ps")
        for fb in range(8):
            nc.tensor.matmul(
                ps[:], wi_g[:, fb, :], h2T_all[:, fb * T:(fb + 1) * T],
                start=(fb == 0), stop=(fb == 7),
            )
        nc.scalar.activation(h3T_all[:, g * T:(g + 1) * T], ps[:],
                             AF.Gelu_apprx_tanh, bias=bi[:, g:g + 1])
        wo_g = wop.tile([128, 512], BF16, tag="wo_g", name="wo_g")
        nc.sync.dma_start(wo_g[:], io["wo"][g * 128:(g + 1) * 128, 0:512])
        for t in range(NT):
            nc.tensor.matmul(
                psE[t][:],
                h3T_all[:, g * T + t * 128: g * T + (t + 1) * 128],
                wo_g[:],
                start=(g == 0), stop=False,
            )
    for t in range(NT):
        nc.tensor.matmul(psE[t][:], ones_row[:], bo_bf[:, 0:512], start=False, stop=True)
        ot = outp.tile([128, 512], F32, tag="ot", name="ot")
        nc.vector.tensor_add(ot[:], psE[t][:], x2_all[:, t * H: t * H + 512])
        nc.sync.dma_start(out_d[t * 128:(t + 1) * 128, 0:512], ot[:])

    # second pass: output columns 512:1024
    psE2 = [wops.tile([128, 512], F32, tag=f"wo_ps{t}", name=f"wo2_ps{t}") for t in range(NT)]
    for g in range(NG):
        wo_g = wop.tile([128, 512], BF16, tag="wo_g", name="wo_g2")
        nc.sync.dma_start(wo_g[:], io["wo"][g * 128:(g + 1) * 128, 512:1024])
        for t in range(NT):
            nc.tensor.matmul(
                psE2[t][:],
                h3T_all[:, g * T + t * 128: g * T + (t + 1) * 128],
                wo_g[:],
                start=(g == 0), stop=False,
            )
    for t in range(NT):
        nc.tensor.matmul(psE2[t][:], ones_row[:], bo_bf[:, 512:1024], start=False, stop=True)
        ot = outp.tile([128, 512], F32, tag="ot", name="ot2")
        nc.vector.tensor_add(ot[:], psE2[t][:], x2_all[:, t * H + 512: t * H + 1024])
        nc.sync.dma_start(out_d[t * 128:(t + 1) * 128, 512:1024], ot[:])

    sD.close()
    s_outer.close()


def _build_program():
    nc = bass.Bass("TRN2", target_bir_lowering=False, debug=False, num_devices=NC)
    io = {}
    io["x"] = nc.dram_tensor("x", [T, H], F32, kind="ExternalInput").ap()
    io["wq"] = nc.dram_tensor("wq", [H, H], BF16, kind="ExternalInput").ap()
    io["wk"] = nc.dram_tensor("wk", [H, H], BF16, kind="ExternalInput").ap()
    io["wv"] = nc.dram_tensor("wv", [H, H], BF16, kind="ExternalInput").ap()
    io["wproj"] = nc.dram_tensor("wproj", [H, H], BF16, kind="ExternalInput").ap()
    io["wi"] = nc.dram_tensor("wi", [FFN // 128, 128, 8, 128], BF16, kind="ExternalInput").ap()
    io["wo"] = nc.dram_tensor("wo", [FFN, H], BF16, kind="ExternalInput").ap()
    io["bq"] = nc.dram_tensor("bq", [128, 8], F32, kind="ExternalInput").ap()
    io["bk"] = nc.dram_tensor("bk", [128, 8], F32, kind="ExternalInput").ap()
    io["bi"] = nc.dram_tensor("bi", [128, 32], F32, kind="ExternalInput").ap()
    io["bv_bf"] = nc.dram_tensor("bv_bf", [1, H], BF16, kind="ExternalInput").ap()
    io["bproj_bf"] = nc.dram_tensor("bproj_bf", [1, H], BF16, kind="ExternalInput").ap()
    io["bo_bf"] = nc.dram_tensor("bo_bf", [1, H], BF16, kind="ExternalInput").ap()
    io["ident"] = nc.dram_tensor("ident", [128, 128], BF16, kind="ExternalInput").ap()
    io["ones_row"] = nc.dram_tensor("ones_row", [1, 128], BF16, kind="ExternalInput").ap()
    io["out"] = nc.dram_tensor("out", [T, H], F32, kind="ExternalOutput").ap()
    with tile.TileContext(nc) as tc:
        _emit(tc, nc, io)
    _split_multiwait(nc)
    return nc


_PROGRAM = None
LAST_RESULTS = None


def kernel(x, ln1_scale, ln1_bias, qkv_w, qkv_b, proj_w, proj_b,
           ln2_scale, ln2_bias, wi_w, wi_b, wo_w, wo_b):
    global _PROGRAM, LAST_RESULTS
    x = np.asarray(x, np.float32)
    ln1_scale = np.asarray(ln1_scale, np.float32); ln1_bias = np.asarray(ln1_bias, np.float32)
    qkv_w = np.asarray(qkv_w, np.float32); qkv_b = np.asarray(qkv_b, np.float32)
    proj_w = np.asarray(proj_w, np.float32); proj_b = np.asarray(proj_b, np.float32)
    ln2_scale = np.asarray(ln2_scale, np.float32); ln2_bias = np.asarray(ln2_bias, np.float32)
    wi_w = np.asarray(wi_w, np.float32); wi_b = np.asarray(wi_b, np.float32)
    wo_w = np.asarray(wo_w, np.float32); wo_b = np.asarray(wo_b, np.float32)

    # fold LN affine params into the next matmul's weights/biases
    qkv_w_eff = ln1_scale[:, None] * qkv_w
    qkv_b_eff = qkv_b + ln1_bias @ qkv_w
    w3 = qkv_w_eff.reshape(H, NH, 3, DH)
    b3 = qkv_b_eff.reshape(NH, 3, DH)
    scale = 1.0 / np.sqrt(np.float32(DH))
    wq = (w3[:, :, 0, :] * scale).reshape(H, H)
    wk = w3[:, :, 1, :].reshape(H, H)
    wv = w3[:, :, 2, :].reshape(H, H)
    bq_v = (b3[:, 0, :] * scale).reshape(H)
    bk_v = b3[:, 1, :].reshape(H)
    bv_v = b3[:, 2, :].reshape(H)
    wi_eff = ln2_scale[:, None] * wi_w
    bi_v = wi_b + ln2_bias @ wi_w

    common = {
        "wq": wq.astype(NPBF16), "wk": wk.astype(NPBF16), "wv": wv.astype(NPBF16),
        "wproj": proj_w.astype(NPBF16),
        "wi": np.ascontiguousarray(
            wi_eff.astype(NPBF16).reshape(8, 128, 32, 128).transpose(2, 1, 0, 3)),
        "wo": wo_w.astype(NPBF16),
        "bq": np.ascontiguousarray(bq_v.reshape(8, 128).T.astype(np.float32)),
        "bk": np.ascontiguousarray(bk_v.reshape(8, 128).T.astype(np.float32)),
        "bi": np.ascontiguousarray(bi_v.reshape(32, 128).T.astype(np.float32)),
        "bv_bf": bv_v.reshape(1, H).astype(NPBF16),
        "bproj_bf": proj_b.reshape(1, H).astype(NPBF16),
        "bo_bf": wo_b.reshape(1, H).astype(NPBF16),
        "ident": np.eye(128, dtype=NPBF16),
        "ones_row": np.ones((1, 128), NPBF16),
    }
    x_flat = x.reshape(B * S, H)
    in_maps = []
    for c in range(NC):
        m = dict(common)
        m["x"] = np.ascontiguousarray(x_flat[c * T:(c + 1) * T, :])
        in_maps.append(m)

    if _PROGRAM is None:
        _PROGRAM = _build_program()
    r = run_bass_kernel_spmd(_PROGRAM, in_maps, list(range(NC)))
    LAST_RESULTS = r
    out = np.concatenate([r.results[c]["out"] for c in range(NC)], axis=0)
    return out.reshape(B, S, H).astype(np.float32)



# revision 7
# speedup vs baseline: 1.1935x; 1.1935x over previous
"""Trainium2 Bass kernel for a pre-LN transformer block (B=2,S=2048,H=1024,NH=16,FFN=4096).

Sharding: 8 cores, 512 tokens/core (4 cores per batch element). K/V are
exchanged within each batch group via fp8 AllGathers (4, pipelined early so
they hide under QKV compute). Attention is software-pipelined around the
scalar-engine exp stream (the true floor); matmuls bf16 with fp32 PSUM;
q/k/v tensors are stored fp8e4 (quantization only perturbs softmax probs and
the tiny attention output, not the residual path).

Self-contained: hardcodes shapes; builds the Bass program once and runs it via
run_bass_kernel_spmd on cores 0-7.
"""

import sys

for _p in ("/root/.axon_site/_ro/trn_rl_repo", "/opt/trn_rl_repo"):
    if _p not in sys.path:
        sys.path.append(_p)

import numpy as np
import ml_dtypes

# If BASS_TRACE is set but the axon NTFF hook module is missing, the trace
# path would crash on import; pre-register a no-op hook shim so tracing
# degrades gracefully instead.
try:
    import antenv.axon_hooks  # noqa: F401
except ImportError:
    import types as _types
    _m = _types.ModuleType("antenv.axon_hooks")
    _m._hook = None
    _m.get_axon_ntff_profile_hook = lambda: _m._hook
    _m.set_axon_ntff_profile_hook = lambda h: setattr(_m, "_hook", h)
    sys.modules["antenv.axon_hooks"] = _m

import bass_rust
import concourse.bass as bass
import concourse.mybir as mybir
import concourse.tile as tile
from concourse.bass_utils import run_bass_kernel_spmd

BF16 = mybir.dt.bfloat16
F32 = mybir.dt.float32
FP8 = mybir.dt.float8e4
AF = mybir.ActivationFunctionType
NPBF16 = np.dtype(ml_dtypes.bfloat16)

B, S, H, NH, DH, FFN = 2, 2048, 1024, 16, 64, 4096
NC = 8                      # cores
T = 512                     # tokens per core
NT = T // 128               # token tiles per core (4)
GROUPS = [[0, 1, 2, 3], [4, 5, 6, 7]]
G = 4                       # cores per batch group
SKEYS = S                   # keys per batch (2048)
NKT = SKEYS // 128          # key tiles (16)
NHP = NH // 2               # head pairs (8)
EPS = 1e-3
VW = DH + 1                 # 65: V columns + ones column per head
# key tiles in half-A (first V AllGather) then half-B order
KT_HALF_A = [g * 4 + j for g in range(4) for j in (0, 1)]
KT_HALF_B = [g * 4 + 2 + j for g in range(4) for j in (0, 1)]

# ---------------------------------------------------------------------------
# Workaround: this walrus build rejects >1 inline sync-wait per instruction.
# After Tile scheduling, move excess waits onto single-wait NoOp carriers
# inserted immediately before the over-limit instruction (same engine, same
# block, so per-engine program order and wait semantics are preserved).
# ---------------------------------------------------------------------------
def _split_multiwait(nc, limit=1):
    n_new = 0
    for f in nc.m.functions:
        for blk in f.blocks:
            insts = blk.instructions
            out = []
            for ins in insts:
                si = getattr(ins, "sync_info", None)
                waits = list(si.on_wait) if si is not None else []
                if len(waits) > limit:
                    for i, w in enumerate(waits[:-limit]):
                        nop = mybir.InstNoOp(
                            name=f"{ins.name}_w{i}",
                            sync_info=mybir.SyncInfo(on_wait=[w], on_update=[]),
                            bass_nofuse=True,
                            engine=ins.engine,
                        )
                        out.append(nop)
                        n_new += 1
                    ins.sync_info = mybir.SyncInfo(
                        on_wait=waits[-limit:], on_update=list(si.on_update)
                    )
                out.append(ins)
            if len(out) != len(insts):
                blk.instructions = out
    return n_new


def _emit(tc, nc, io):
    """Emit the per-core program. io: dict of DRAM APs."""
    from contextlib import ExitStack

    x_d = io["x"]
    out_d = io["out"]

    s_outer = ExitStack()

    constp = s_outer.enter_context(tc.tile_pool(name="constp", bufs=1))
    dramp = s_outer.enter_context(tc.tile_pool(name="dramp", bufs=1, space="DRAM"))

    # constants / biases
    ident = constp.tile([128, 128], BF16)
    nc.sync.dma_start(ident[:], io["ident"][:])
    ones_row = constp.tile([1, 128], BF16)
    nc.sync.dma_start(ones_row[:], io["ones_row"][:])
    ones_f32 = constp.tile([1, 128], F32)
    nc.sync.dma_start(ones_f32[:], io["ones_f32"][:])
    bq = constp.tile([128, 8], F32); nc.sync.dma_start(bq[:], io["bq"][:])
    bk = constp.tile([128, 8], F32); nc.sync.dma_start(bk[:], io["bk"][:])
    bi = constp.tile([128, 32], F32); nc.sync.dma_start(bi[:], io["bi"][:])
    bv_bf = constp.tile([1, H], BF16); nc.sync.dma_start(bv_bf[:], io["bv_bf"][:])
    bproj_bf = constp.tile([1, H], BF16); nc.sync.dma_start(bproj_bf[:], io["bproj_bf"][:])
    bo_bf = constp.tile([1, H], BF16); nc.sync.dma_start(bo_bf[:], io["bo_bf"][:])
    eps_t = constp.tile([128, 1], F32); nc.gpsimd.memset(eps_t[:], float(EPS))

    # collective buffers (fp8: half the wire/DRAM traffic of bf16)
    cc_k_in_a = dramp.tile([512, T], FP8)
    cc_k_in_b = dramp.tile([512, T], FP8)
    cc_k_out_a = dramp.tile([G * 512, T], FP8)
    cc_k_out_b = dramp.tile([G * 512, T], FP8)
    cc_v_in_a = dramp.tile([256, H], FP8)
    cc_v_in_b = dramp.tile([256, H], FP8)
    cc_v_out_a = dramp.tile([G * 256, H], FP8)
    cc_v_out_b = dramp.tile([G * 256, H], FP8)

    # persistent activations
    persp = s_outer.enter_context(tc.tile_pool(name="persp", bufs=1))
    x_all = persp.tile([128, NT * H], F32, name="x_all")
    qT_all = persp.tile([128, 8 * T], FP8, name="qT_all")
    ctxT_all = persp.tile([128, 8 * T], BF16, name="ctxT_all")
    wproj_sb = persp.tile([128, 8 * H], BF16, name="wproj_sb")

    def layer_norm_to(pool, h_out_slice, x_slice):
        """x_slice [128,H] f32 -> h_out_slice [128,H] standardized."""
        sq = pool.tile([128, H], F32, tag="ln_sq")
        nsum = pool.tile([128, 1], F32, tag="ln_nsum")
        s2 = pool.tile([128, 1], F32, tag="ln_s2")
        var = pool.tile([128, 1], F32, tag="ln_var")
        std = pool.tile([128, 1], F32, tag="ln_std")
        rs = pool.tile([128, 1], F32, tag="ln_rs")
        nmu = pool.tile([128, 1], F32, tag="ln_nmu")
        nmurs = pool.tile([128, 1], F32, tag="ln_nmurs")
        nc.vector.reduce_sum(nsum[:], x_slice, axis=mybir.AxisListType.X, negate=True)
        nc.vector.tensor_mul(sq[:], x_slice, x_slice)
        nc.vector.reduce_sum(s2[:], sq[:], axis=mybir.AxisListType.X)
        nc.vector.tensor_scalar_mul(nmu[:], nsum[:], 1.0 / H)      # -mean
        nc.vector.tensor_scalar_mul(s2[:], s2[:], 1.0 / H)         # E[x^2]
        nc.vector.tensor_mul(var[:], nmu[:], nmu[:])               # mean^2
        nc.vector.tensor_sub(var[:], s2[:], var[:])                # var
        nc.scalar.activation(std[:], var[:], AF.Sqrt, bias=eps_t[:])
        nc.vector.reciprocal(rs[:], std[:])
        nc.vector.tensor_mul(nmurs[:], nmu[:], rs[:])              # -mean*rs
        nc.scalar.activation(h_out_slice, x_slice, AF.Identity, bias=nmurs[:], scale=rs[:])

    def transpose_128(dst_slice, src_slice, tps):
        """PE-transpose src [128,128] bf16 -> dst [128,128] bf16."""
        ps = tps.tile([128, 128], BF16, tag="tp")
        nc.tensor.transpose(ps[:], src_slice, ident[:])
        nc.vector.tensor_copy(dst_slice, ps[:])

    # =====================================================================
    # Phase A: load x, LN1, h1^T; k^T -> AG(Ka..), v -> AG(Va,Vb), AG(Kb);
    # q^T interleaved so head pair 0 can start scoring as soon as Ka lands.
    # =====================================================================
    sA = ExitStack()
    wq_p = sA.enter_context(tc.tile_pool(name="wq_p", bufs=2))
    lnp = sA.enter_context(tc.tile_pool(name="lnp", bufs=2))
    h1p = sA.enter_context(tc.tile_pool(name="h1p", bufs=1))
    h1Tp = sA.enter_context(tc.tile_pool(name="h1Tp", bufs=1))
    tpsA = sA.enter_context(tc.tile_pool(name="tpsA", bufs=2, space="PSUM"))
    mmpsA = sA.enter_context(tc.tile_pool(name="mmpsA", bufs=2, space="PSUM"))
    stgA = sA.enter_context(tc.tile_pool(name="stgA", bufs=4))

    h1_all = h1p.tile([128, NT * H], BF16)
    h1T_all = h1Tp.tile([128, 8 * T], BF16)

    # x + wk first so the K^T -> AllGather chain starts as early as possible
    for t in range(NT):
        nc.sync.dma_start(x_all[:, t * H:(t + 1) * H], x_d[t * 128:(t + 1) * 128, :])
    wk_sb = wq_p.tile([128, 8 * H], BF16, tag="w3", name="wk_sb")
    for fb in range(8):
        nc.sync.dma_start(wk_sb[:, fb * H:(fb + 1) * H], io["wk"][fb * 128:(fb + 1) * 128, :])
    wq_sb = wq_p.tile([128, 8 * H], BF16, tag="w3", name="wq_sb")
    for fb in range(8):
        nc.sync.dma_start(wq_sb[:, fb * H:(fb + 1) * H], io["wq"][fb * 128:(fb + 1) * 128, :])

    for t in range(NT):
        layer_norm_to(lnp, h1_all[:, t * H:(t + 1) * H], x_all[:, t * H:(t + 1) * H])
        for fb in range(8):
            transpose_128(
                h1T_all[:, fb * T + t * 128: fb * T + (t + 1) * 128],
                h1_all[:, t * H + fb * 128: t * H + (fb + 1) * 128],
                tpsA,
            )

    # k^T feature-major: [128 feats, T] per col-tile, written fp8
    def emit_kt(ct):
        ps = mmpsA.tile([128, T], F32, tag="mm_qk")
        for fb in range(8):
            nc.tensor.matmul(
                ps[:],
                wk_sb[:, fb * H + ct * 128: fb * H + (ct + 1) * 128],
                h1T_all[:, fb * T:(fb + 1) * T],
                start=(fb == 0), stop=(fb == 7),
            )
        ktmp = stgA.tile([128, T], FP8, tag="ktmp")
        nc.scalar.activation(ktmp[:], ps[:], AF.Identity, bias=bk[:, ct:ct + 1])
        dst = cc_k_in_a if ct < 4 else cc_k_in_b
        nc.sync.dma_start(dst[(ct % 4) * 128:(ct % 4 + 1) * 128, :], ktmp[:])

    def emit_qt(ct):
        ps = mmpsA.tile([128, T], F32, tag="mm_qk")
        for fb in range(8):
            nc.tensor.matmul(
                ps[:],
                wq_sb[:, fb * H + ct * 128: fb * H + (ct + 1) * 128],
                h1T_all[:, fb * T:(fb + 1) * T],
                start=(fb == 0), stop=(fb == 7),
            )
        nc.scalar.activation(
            qT_all[:, ct * T:(ct + 1) * T], ps[:], AF.Identity,
            bias=bq[:, ct:ct + 1])

    for ct in range(4):
        emit_kt(ct)
    nc.gpsimd.collective_compute(
        "AllGather", mybir.AluOpType.bypass, replica_groups=GROUPS,
        ins=[cc_k_in_a.opt()], outs=[cc_k_out_a.opt()],
    )
    for ct in range(4, 8):
        emit_kt(ct)
    emit_qt(0)

    # v token-major: [128 tok, H] fp8 (wv reuses the wk slot once kT is done)
    wv_sb = wq_p.tile([128, 8 * H], BF16, tag="w3", name="wv_sb")
    for fb in range(8):
        nc.sync.dma_start(wv_sb[:, fb * H:(fb + 1) * H], io["wv"][fb * 128:(fb + 1) * 128, :])

    def emit_v(t):
        for cc in range(2):
            ps = mmpsA.tile([128, 512], F32, tag="mm_v")
            for fb in range(8):
                nc.tensor.matmul(
                    ps[:],
                    h1T_all[:, fb * T + t * 128: fb * T + (t + 1) * 128],
                    wv_sb[:, fb * H + cc * 512: fb * H + (cc + 1) * 512],
                    start=(fb == 0), stop=False,
                )
            nc.tensor.matmul(ps[:], ones_row[:], bv_bf[:, cc * 512:(cc + 1) * 512],
                             start=False, stop=True)
            vtmp = stgA.tile([128, 512], FP8, tag="vtmp")
            nc.vector.tensor_copy(vtmp[:], ps[:])
            dst = cc_v_in_a if t < 2 else cc_v_in_b
            nc.sync.dma_start(dst[(t % 2) * 128:(t % 2 + 1) * 128, cc * 512:(cc + 1) * 512], vtmp[:])

    emit_v(0); emit_v(1)
    nc.gpsimd.collective_compute(
        "AllGather", mybir.AluOpType.bypass, replica_groups=GROUPS,
        ins=[cc_v_in_a.opt()], outs=[cc_v_out_a.opt()],
    )
    emit_v(2); emit_v(3)
    nc.gpsimd.collective_compute(
        "AllGather", mybir.AluOpType.bypass, replica_groups=GROUPS,
        ins=[cc_v_in_b.opt()], outs=[cc_v_out_b.opt()],
    )
    nc.gpsimd.collective_compute(
        "AllGather", mybir.AluOpType.bypass, replica_groups=GROUPS,
        ins=[cc_k_in_b.opt()], outs=[cc_k_out_b.opt()],
    )

    for ct in range(1, 8):
        emit_qt(ct)

    sA.close()

    # prefetch proj weights during attention
    for hp in range(8):
        nc.sync.dma_start(wproj_sb[:, hp * H:(hp + 1) * H],
                          io["wproj"][hp * 128:(hp + 1) * 128, :])

    # =====================================================================
    # Phase B: attention, software-pipelined around the ACT exp stream.
    # scores^T per key-tile (row-packed head pairs, fp8 q/k), exp on ACT into
    # a deep pb ring, ctx^T via V'=[V|ones] (M=65, bf16), tail normalize via
    # DVE reciprocal_approx_fast + GpSimd partition_broadcast.
    # =====================================================================
    sB = ExitStack()
    vstg = sB.enter_context(tc.tile_pool(name="vstg", bufs=4))
    kpool = sB.enter_context(tc.tile_pool(name="kpool", bufs=4))
    spool = sB.enter_context(tc.tile_pool(name="spool", bufs=2, space="PSUM"))
    cpool = sB.enter_context(tc.tile_pool(name="cpool", bufs=4, space="PSUM"))
    ppool = sB.enter_context(tc.tile_pool(name="ppool", bufs=24))
    rpool = sB.enter_context(tc.tile_pool(name="rpool", bufs=2))
    vsb = sB.enter_context(tc.tile_pool(name="vsb_p", bufs=1)).tile([128, NKT * NH * VW], BF16, name="vsb")

    def load_kt(hp):
        kt_hp = kpool.tile([128, SKEYS], FP8, tag="kt_hp", name="kt_hp")
        cko, hpo = (cc_k_out_a, hp) if hp < 4 else (cc_k_out_b, hp - 4)
        for g in range(G):
            nc.sync.dma_start(kt_hp[:, g * T:(g + 1) * T],
                              cko[g * 512 + hpo * 128: g * 512 + (hpo + 1) * 128, :])
        return kt_hp

    # ones columns for all key tiles up front (DVE; no data deps)
    for kt in range(NKT):
        blk = vsb[:, kt * NH * VW:(kt + 1) * NH * VW]
        nc.vector.memset(blk.rearrange("p (h x) -> p h x", x=VW)[:, :, DH:VW], 1.0)

    def load_v_half(half_kts, cc_v_out):
        # V from AllGather output (fp8 -> bf16), interleaving ones per head
        for kt in half_kts:
            g, j = kt // 4, (kt % 4) % 2
            vplain = vstg.tile([128, H], FP8, tag="vplain", name="vplain")
            nc.sync.dma_start(vplain[:], cc_v_out[g * 256 + j * 128: g * 256 + (j + 1) * 128, :])
            blk = vsb[:, kt * NH * VW:(kt + 1) * NH * VW]
            dst = blk.rearrange("p (h x) -> p h x", x=VW)
            nc.vector.tensor_copy(dst[:, :, 0:DH], vplain[:].rearrange("p (h d) -> p h d", d=DH))

    kt_tiles = [None] * NHP
    for hp in range(2):
        kt_tiles[hp] = load_kt(hp)
    load_v_half(KT_HALF_A, cc_v_out_a)
    load_v_half(KT_HALF_B, cc_v_out_b)
    for hp in range(2, NHP):
        kt_tiles[hp] = load_kt(hp)

    KT_ORDER = KT_HALF_A + KT_HALF_B
    pb_tiles = {}   # (hp, kt) -> pb tile (live between exp and ctx)
    cps_tiles = {}  # hp -> (cps0, cps1)

    def emit_scores(hp, kts):
        """scores + exp for the given key tiles of head pair hp."""
        kt_hp = kt_tiles[hp]
        for kt in kts:
            ps = spool.tile([128, 1024], F32, tag="ps", name="ps")
            nc.tensor.matmul(
                ps[:, 0:512],
                kt_hp[0:64, kt * 128:(kt + 1) * 128],
                qT_all[0:64, hp * T:(hp + 1) * T],
                start=True, stop=True, tile_position=(0, 0),
            )
            nc.tensor.matmul(
                ps[:, 512:1024],
                kt_hp[64:128, kt * 128:(kt + 1) * 128],
                qT_all[64:128, hp * T:(hp + 1) * T],
                start=True, stop=True, tile_position=(64, 0),
            )
            pb = ppool.tile([128, 1024], BF16, tag="pb", name="pb")
            nc.scalar.activation(pb[:], ps[:], AF.Exp)
            pb_tiles[(hp, kt)] = pb

    def emit_ctx(hp, kts, first, last):
        if first:
            cps_tiles[hp] = (
                cpool.tile([128, T], F32, tag="cps", name=f"cps0_{hp}"),
                cpool.tile([128, T], F32, tag="cps", name=f"cps1_{hp}"),
            )
        cps0, cps1 = cps_tiles[hp]
        for i, kt in enumerate(kts):
            pb = pb_tiles.pop((hp, kt))
            st = first and i == 0
            sp = last and i == len(kts) - 1
            for h, cps in enumerate((cps0, cps1)):
                head = hp * 2 + h
                nc.tensor.matmul(
                    cps[0:VW, :],
                    vsb[:, kt * NH * VW + head * VW: kt * NH * VW + (head + 1) * VW],
                    pb[:, h * 512:(h + 1) * 512],
                    start=st, stop=sp,
                )

    def emit_fin(hp):
        """normalize ctx^T by 1/sumexp; writes ctxT_all (bf16)."""
        cps0, cps1 = cps_tiles.pop(hp)
        rc0 = rpool.tile([1, T], F32, tag="rc0")
        rc1 = rpool.tile([1, T], F32, tag="rc1")
        nc.vector.reciprocal(rc0[:], cps0[DH:VW, :])
        nc.vector.reciprocal(rc1[:], cps1[DH:VW, :])
        # broadcast 1/sumexp across 64 partitions per head via K=1 f32 matmuls
        bb = spool.tile([128, T], F32, tag="ps", name=f"bb_{hp}")
        nc.tensor.matmul(bb[0:64, :], ones_f32[:, 0:64], rc0[:],
                         start=True, stop=True, tile_position=(0, 0))
        nc.tensor.matmul(bb[64:128, :], ones_f32[:, 0:64], rc1[:],
                         start=True, stop=True, tile_position=(0, 64))
        rb = rpool.tile([128, T], F32, tag="rb")
        nc.vector.tensor_copy(rb[:], bb[:])
        nc.vector.tensor_mul(ctxT_all[0:64, hp * T:(hp + 1) * T], cps0[0:DH, :], rb[0:64, :])
        nc.vector.tensor_mul(ctxT_all[64:128, hp * T:(hp + 1) * T], cps1[0:DH, :], rb[64:128, :])

    # schedule: scores run one head pair ahead of ctx so the PE never waits
    # on the exp stream; ctx(hp) consumes pb tiles well after exp produced them
    emit_scores(0, KT_ORDER)
    emit_scores(1, KT_ORDER)
    for hp in range(NHP):
        emit_ctx(hp, KT_HALF_A, first=True, last=False)
        emit_ctx(hp, KT_HALF_B, first=False, last=True)
        if hp + 2 < NHP:
            emit_scores(hp + 2, KT_ORDER)
        emit_fin(hp)

    sB.close()

    # =====================================================================
    # Phase C: proj (token-major) + residual -> x2, LN2 -> h2^T
    # =====================================================================
    sCD = ExitStack()
    x2p = sCD.enter_context(tc.tile_pool(name="x2p", bufs=1))
    h2Tp = sCD.enter_context(tc.tile_pool(name="h2Tp", bufs=1))
    h3Tp = sCD.enter_context(tc.tile_pool(name="h3Tp", bufs=1))
    x2_all = x2p.tile([128, NT * H], F32, name="x2_all")
    h2T_all = h2Tp.tile([128, 8 * T], BF16, name="h2T_all")
    h3T_all = h3Tp.tile([128, 32 * T], BF16, name="h3T_all")

    sC = ExitStack()
    lnp2 = sC.enter_context(tc.tile_pool(name="lnp2", bufs=2))
    h2p = sC.enter_context(tc.tile_pool(name="h2p", bufs=1))
    tpsC = sC.enter_context(tc.tile_pool(name="tpsC", bufs=2, space="PSUM"))
    mmpsC = sC.enter_context(tc.tile_pool(name="mmpsC", bufs=2, space="PSUM"))
    stgC = sC.enter_context(tc.tile_pool(name="stgC", bufs=4))

    h2_all = h2p.tile([128, NT * H], BF16)

    for t in range(NT):
        for cc in range(2):
            ps = mmpsC.tile([128, 512], F32, tag="pj")
            for hp in range(8):
                nc.tensor.matmul(
                    ps[:],
                    ctxT_all[:, hp * T + t * 128: hp * T + (t + 1) * 128],
                    wproj_sb[:, hp * H + cc * 512: hp * H + (cc + 1) * 512],
                    start=(hp == 0), stop=False,
                )
            nc.tensor.matmul(ps[:], ones_row[:], bproj_bf[:, cc * 512:(cc + 1) * 512],
                             start=False, stop=True)
            nc.vector.tensor_add(
                x2_all[:, t * H + cc * 512: t * H + (cc + 1) * 512],
                ps[:], x_all[:, t * H + cc * 512: t * H + (cc + 1) * 512])
        layer_norm_to(lnp2, h2_all[:, t * H:(t + 1) * H], x2_all[:, t * H:(t + 1) * H])
        for fb in range(8):
            transpose_128(
                h2T_all[:, fb * T + t * 128: fb * T + (t + 1) * 128],
                h2_all[:, t * H + fb * 128: t * H + (fb + 1) * 128],
                tpsC,
            )

    sC.close()

    # =====================================================================
    # Phase D+E fused: per g-tile: wi matmuls + gelu -> h3T[g], then wo
    # matmuls for output columns 0:512 accumulate into 4 persistent psums.
    # Second pass re-reads h3T for output columns 512:1024.
    # =====================================================================
    sD = ExitStack()
    wip = sD.enter_context(tc.tile_pool(name="wip", bufs=6))
    wop = sD.enter_context(tc.tile_pool(name="wop", bufs=6))
    mmpsD = sD.enter_context(tc.tile_pool(name="mmpsD", bufs=4, space="PSUM"))
    wops = sD.enter_context(tc.tile_pool(name="wops", bufs=1, space="PSUM"))
    outp = sD.enter_context(tc.tile_pool(name="outp", bufs=2))

    NG = FFN // 128  # 32
    psE = [wops.tile([128, 512], F32, tag=f"wo_ps{t}", name=f"wo_ps{t}") for t in range(NT)]
    for g in range(NG):
        wi_g = wip.tile([128, 8, 128], BF16, tag="wi_g", name="wi_g")
        src = io["wi"][g:g + 1, :, :, :].rearrange("o p f c -> (o p) f c")
        nc.sync.dma_start(wi_g[:], src)
        ps = mmpsD.tile([128, T], F32, tag="wi_ps", name="wi_ps")
        for fb in range(8):
            nc.tensor.matmul(
                ps[:], wi_g[:, fb, :], h2T_all[:, fb * T:(fb + 1) * T],
                start=(fb == 0), stop=(fb == 7),
            )
        nc.scalar.activation(h3T_all[:, g * T:(g + 1) * T], ps[:],
                             AF.Gelu_apprx_tanh, bias=bi[:, g:g + 1])
        wo_g = wop.tile([128, 512], BF16, tag="wo_g", name="wo_g")
        nc.sync.dma_start(wo_g[:], io["wo"][g * 128:(g + 1) * 128, 0:512])
        for t in range(NT):
            nc.tensor.matmul(
                psE[t][:],
                h3T_all[:, g * T + t * 128: g * T + (t + 1) * 128],
                wo_g[:],
                start=(g == 0), stop=False,
            )
    for t in range(NT):
        nc.tensor.matmul(psE[t][:], ones_row[:], bo_bf[:, 0:512], start=False, stop=True)
        ot = outp.tile([128, 512], F32, tag="ot", name="ot")
        nc.vector.tensor_add(ot[:], psE[t][:], x2_all[:, t * H: t * H + 512])
        nc.sync.dma_start(out_d[t * 128:(t + 1) * 128, 0:512], ot[:])

    # second pass: output columns 512:1024
    psE2 = [wops.tile([128, 512], F32, tag=f"wo_ps{t}", name=f"wo2_ps{t}") for t in range(NT)]
    for g in range(NG):
        wo_g = wop.tile([128, 512], BF16, tag="wo_g", name="wo_g2")
        nc.sync.dma_start(wo_g[:], io["wo"][g * 128:(g + 1) * 128, 512:1024])
        for t in range(NT):
            nc.tensor.matmul(
                psE2[t][:],
                h3T_all[:, g * T + t * 128: g * T + (t + 1) * 128],
                wo_g[:],
                start=(g == 0), stop=False,
            )
    for t in range(NT):
        nc.tensor.matmul(psE2[t][:], ones_row[:], bo_bf[:, 512:1024], start=False, stop=True)
        ot = outp.tile([128, 512], F32, tag="ot", name="ot2")
        nc.vector.tensor_add(ot[:], psE2[t][:], x2_all[:, t * H + 512: t * H + 1024])
        nc.sync.dma_start(out_d[t * 128:(t + 1) * 128, 512:1024], ot[:])

    sD.close()
    sCD.close()
    s_outer.close()


def _build_program():
    nc = bass.Bass("TRN2", target_bir_lowering=False, debug=False, num_devices=NC)
    io = {}
    io["x"] = nc.dram_tensor("x", [T, H], F32, kind="ExternalInput").ap()
    io["wq"] = nc.dram_tensor("wq", [H, H], BF16, kind="ExternalInput").ap()
    io["wk"] = nc.dram_tensor("wk", [H, H], BF16, kind="ExternalInput").ap()
    io["wv"] = nc.dram_tensor("wv", [H, H], BF16, kind="ExternalInput").ap()
    io["wproj"] = nc.dram_tensor("wproj", [H, H], BF16, kind="ExternalInput").ap()
    io["wi"] = nc.dram_tensor("wi", [FFN // 128, 128, 8, 128], BF16, kind="ExternalInput").ap()
    io["wo"] = nc.dram_tensor("wo", [FFN, H], BF16, kind="ExternalInput").ap()
    io["bq"] = nc.dram_tensor("bq", [128, 8], F32, kind="ExternalInput").ap()
    io["bk"] = nc.dram_tensor("bk", [128, 8], F32, kind="ExternalInput").ap()
    io["bi"] = nc.dram_tensor("bi", [128, 32], F32, kind="ExternalInput").ap()
    io["bv_bf"] = nc.dram_tensor("bv_bf", [1, H], BF16, kind="ExternalInput").ap()
    io["bproj_bf"] = nc.dram_tensor("bproj_bf", [1, H], BF16, kind="ExternalInput").ap()
    io["bo_bf"] = nc.dram_tensor("bo_bf", [1, H], BF16, kind="ExternalInput").ap()
    io["ident"] = nc.dram_tensor("ident", [128, 128], BF16, kind="ExternalInput").ap()
    io["ones_row"] = nc.dram_tensor("ones_row", [1, 128], BF16, kind="ExternalInput").ap()
    io["ones_f32"] = nc.dram_tensor("ones_f32", [1, 128], F32, kind="ExternalInput").ap()
    io["out"] = nc.dram_tensor("out", [T, H], F32, kind="ExternalOutput").ap()
    with tile.TileContext(nc) as tc:
        _emit(tc, nc, io)
    _split_multiwait(nc)
    return nc


_PROGRAM = None
LAST_RESULTS = None


def kernel(x, ln1_scale, ln1_bias, qkv_w, qkv_b, proj_w, proj_b,
           ln2_scale, ln2_bias, wi_w, wi_b, wo_w, wo_b):
    global _PROGRAM, LAST_RESULTS
    x = np.asarray(x, np.float32)
    ln1_scale = np.asarray(ln1_scale, np.float32); ln1_bias = np.asarray(ln1_bias, np.float32)
    qkv_w = np.asarray(qkv_w, np.float32); qkv_b = np.asarray(qkv_b, np.float32)
    proj_w = np.asarray(proj_w, np.float32); proj_b = np.asarray(proj_b, np.float32)
    ln2_scale = np.asarray(ln2_scale, np.float32); ln2_bias = np.asarray(ln2_bias, np.float32)
    wi_w = np.asarray(wi_w, np.float32); wi_b = np.asarray(wi_b, np.float32)
    wo_w = np.asarray(wo_w, np.float32); wo_b = np.asarray(wo_b, np.float32)

    # fold LN affine params into the next matmul's weights/biases
    qkv_w_eff = ln1_scale[:, None] * qkv_w
    qkv_b_eff = qkv_b + ln1_bias @ qkv_w
    w3 = qkv_w_eff.reshape(H, NH, 3, DH)
    b3 = qkv_b_eff.reshape(NH, 3, DH)
    scale = 1.0 / np.sqrt(np.float32(DH))
    wq = (w3[:, :, 0, :] * scale).reshape(H, H)
    wk = w3[:, :, 1, :].reshape(H, H)
    wv = w3[:, :, 2, :].reshape(H, H)
    bq_v = (b3[:, 0, :] * scale).reshape(H)
    bk_v = b3[:, 1, :].reshape(H)
    bv_v = b3[:, 2, :].reshape(H)
    wi_eff = ln2_scale[:, None] * wi_w
    bi_v = wi_b + ln2_bias @ wi_w

    common = {
        "wq": wq.astype(NPBF16), "wk": wk.astype(NPBF16), "wv": wv.astype(NPBF16),
        "wproj": proj_w.astype(NPBF16),
        "wi": np.ascontiguousarray(
            wi_eff.astype(NPBF16).reshape(8, 128, 32, 128).transpose(2, 1, 0, 3)),
        "wo": wo_w.astype(NPBF16),
        "bq": np.ascontiguousarray(bq_v.reshape(8, 128).T.astype(np.float32)),
        "bk": np.ascontiguousarray(bk_v.reshape(8, 128).T.astype(np.float32)),
        "bi": np.ascontiguousarray(bi_v.reshape(32, 128).T.astype(np.float32)),
        "bv_bf": bv_v.reshape(1, H).astype(NPBF16),
        "bproj_bf": proj_b.reshape(1, H).astype(NPBF16),
        "bo_bf": wo_b.reshape(1, H).astype(NPBF16),
        "ident": np.eye(128, dtype=NPBF16),
        "ones_row": np.ones((1, 128), NPBF16),
        "ones_f32": np.ones((1, 128), np.float32),
    }
    x_flat = x.reshape(B * S, H)
    in_maps = []
    for c in range(NC):
        m = dict(common)
        m["x"] = np.ascontiguousarray(x_flat[c * T:(c + 1) * T, :])
        in_maps.append(m)

    if _PROGRAM is None:
        _PROGRAM = _build_program()
    r = run_bass_kernel_spmd(_PROGRAM, in_maps, list(range(NC)))
    LAST_RESULTS = r
    out = np.concatenate([r.results[c]["out"] for c in range(NC)], axis=0)
    return out.reshape(B, S, H).astype(np.float32)


# revision 9
# speedup vs baseline: 1.2013x; 1.0065x over previous
"""Trainium2 Bass kernel for a pre-LN transformer block (B=2,S=2048,H=1024,NH=16,FFN=4096).

Sharding: 8 cores, 512 tokens/core (4 cores per batch element). K/V are
exchanged within each batch group via fp8 AllGathers, triggered early so they
hide under QKV compute. Attention is software-pipelined around the scalar
engine's exp stream (the hard floor): scores run two head-pairs ahead of
context, probs live in a deep fp8 ring, context is computed token-major
(half the PE stream cycles of the feat-major form, and the softmax
normalization becomes a per-partition scalar multiply).

Self-contained: hardcodes shapes; builds the Bass program once and runs it via
run_bass_kernel_spmd on cores 0-7.
"""

import sys

for _p in ("/root/.axon_site/_ro/trn_rl_repo", "/opt/trn_rl_repo"):
    if _p not in sys.path:
        sys.path.append(_p)

import numpy as np
import ml_dtypes

# If BASS_TRACE is set but the axon NTFF hook module is missing, the trace
# path would crash on import; pre-register a no-op hook shim so tracing
# degrades gracefully instead.
try:
    import antenv.axon_hooks  # noqa: F401
except ImportError:
    import types as _types
    _m = _types.ModuleType("antenv.axon_hooks")
    _m._hook = None
    _m.get_axon_ntff_profile_hook = lambda: _m._hook
    _m.set_axon_ntff_profile_hook = lambda h: setattr(_m, "_hook", h)
    sys.modules["antenv.axon_hooks"] = _m

import bass_rust
import concourse.bass as bass
import concourse.mybir as mybir
import concourse.tile as tile
from concourse.bass_utils import run_bass_kernel_spmd

BF16 = mybir.dt.bfloat16
F32 = mybir.dt.float32
FP8 = mybir.dt.float8e4
AF = mybir.ActivationFunctionType
NPBF16 = np.dtype(ml_dtypes.bfloat16)

B, S, H, NH, DH, FFN = 2, 2048, 1024, 16, 64, 4096
NC = 8                      # cores
T = 512                     # tokens per core
NT = T // 128               # token tiles per core (4)
GROUPS = [[0, 1, 2, 3], [4, 5, 6, 7]]
G = 4                       # cores per batch group
SKEYS = S                   # keys per batch (2048)
NKT = SKEYS // 128          # key tiles (16)
NHP = NH // 2               # head pairs (8)
EPS = 1e-3
VW = DH + 1                 # 65: V columns + ones column per head
# key tiles in half-A (first V AllGather) then half-B order
KT_HALF_A = [g * 4 + j for g in range(4) for j in (0, 1)]
KT_HALF_B = [g * 4 + 2 + j for g in range(4) for j in (0, 1)]

# ---------------------------------------------------------------------------
# Workaround: this walrus build rejects >1 inline sync-wait per instruction.
# After Tile scheduling, move excess waits onto single-wait NoOp carriers
# inserted immediately before the over-limit instruction (same engine, same
# block, so per-engine program order and wait semantics are preserved).
# ---------------------------------------------------------------------------
def _split_multiwait(nc, limit=1):
    n_new = 0
    for f in nc.m.functions:
        for blk in f.blocks:
            insts = blk.instructions
            out = []
            for ins in insts:
                si = getattr(ins, "sync_info", None)
                waits = list(si.on_wait) if si is not None else []
                if len(waits) > limit:
                    for i, w in enumerate(waits[:-limit]):
                        nop = mybir.InstNoOp(
                            name=f"{ins.name}_w{i}",
                            sync_info=mybir.SyncInfo(on_wait=[w], on_update=[]),
                            bass_nofuse=True,
                            engine=ins.engine,
                        )
                        out.append(nop)
                        n_new += 1
                    ins.sync_info = mybir.SyncInfo(
                        on_wait=waits[-limit:], on_update=list(si.on_update)
                    )
                out.append(ins)
            if len(out) != len(insts):
                blk.instructions = out
    return n_new


def _emit(tc, nc, io):
    """Emit the per-core program. io: dict of DRAM APs."""
    from contextlib import ExitStack

    x_d = io["x"]
    out_d = io["out"]

    s_outer = ExitStack()

    constp = s_outer.enter_context(tc.tile_pool(name="constp", bufs=1))
    dramp = s_outer.enter_context(tc.tile_pool(name="dramp", bufs=1, space="DRAM"))

    # constants / biases
    ident = constp.tile([128, 128], BF16)
    nc.sync.dma_start(ident[:], io["ident"][:])
    ones_row = constp.tile([1, 128], BF16)
    nc.sync.dma_start(ones_row[:], io["ones_row"][:])
    bq = constp.tile([128, 8], F32); nc.sync.dma_start(bq[:], io["bq"][:])
    bk = constp.tile([128, 8], F32); nc.sync.dma_start(bk[:], io["bk"][:])
    bi = constp.tile([128, 32], F32); nc.sync.dma_start(bi[:], io["bi"][:])
    bv_bf = constp.tile([1, H], BF16); nc.sync.dma_start(bv_bf[:], io["bv_bf"][:])
    bproj_bf = constp.tile([1, H], BF16); nc.sync.dma_start(bproj_bf[:], io["bproj_bf"][:])
    bo_bf = constp.tile([1, H], BF16); nc.sync.dma_start(bo_bf[:], io["bo_bf"][:])
    eps_t = constp.tile([128, 1], F32); nc.gpsimd.memset(eps_t[:], float(EPS))

    # collective buffers (fp8: half the wire/DRAM traffic of bf16)
    cc_k_in_a = dramp.tile([512, T], FP8)
    cc_k_in_b = dramp.tile([512, T], FP8)
    cc_k_out_a = dramp.tile([G * 512, T], FP8)
    cc_k_out_b = dramp.tile([G * 512, T], FP8)
    cc_v_in_a = dramp.tile([256, H], FP8)
    cc_v_in_b = dramp.tile([256, H], FP8)
    cc_v_out_a = dramp.tile([G * 256, H], FP8)
    cc_v_out_b = dramp.tile([G * 256, H], FP8)

    # persistent activations
    persp = s_outer.enter_context(tc.tile_pool(name="persp", bufs=1))
    x_all = persp.tile([128, NT * H], F32, name="x_all")
    qT_all = persp.tile([128, 8 * T], FP8, name="qT_all")
    ctxT_all = persp.tile([128, 8 * T], BF16, name="ctxT_all")
    wproj_sb = persp.tile([128, 8 * H], BF16, name="wproj_sb")

    def layer_norm_to(pool, h_out_slice, x_slice):
        """x_slice [128,H] f32 -> h_out_slice [128,H] standardized."""
        sq = pool.tile([128, H], F32, tag="ln_sq")
        nsum = pool.tile([128, 1], F32, tag="ln_nsum")
        s2 = pool.tile([128, 1], F32, tag="ln_s2")
        var = pool.tile([128, 1], F32, tag="ln_var")
        std = pool.tile([128, 1], F32, tag="ln_std")
        rs = pool.tile([128, 1], F32, tag="ln_rs")
        nmu = pool.tile([128, 1], F32, tag="ln_nmu")
        nmurs = pool.tile([128, 1], F32, tag="ln_nmurs")
        nc.vector.reduce_sum(nsum[:], x_slice, axis=mybir.AxisListType.X, negate=True)
        nc.vector.tensor_mul(sq[:], x_slice, x_slice)
        nc.vector.reduce_sum(s2[:], sq[:], axis=mybir.AxisListType.X)
        nc.vector.tensor_scalar_mul(nmu[:], nsum[:], 1.0 / H)      # -mean
        nc.vector.tensor_scalar_mul(s2[:], s2[:], 1.0 / H)         # E[x^2]
        nc.vector.tensor_mul(var[:], nmu[:], nmu[:])               # mean^2
        nc.vector.tensor_sub(var[:], s2[:], var[:])                # var
        nc.scalar.activation(std[:], var[:], AF.Sqrt, bias=eps_t[:])
        nc.vector.reciprocal(rs[:], std[:])
        nc.vector.tensor_mul(nmurs[:], nmu[:], rs[:])              # -mean*rs
        nc.scalar.activation(h_out_slice, x_slice, AF.Identity, bias=nmurs[:], scale=rs[:])

    def transpose_128(dst_slice, src_slice, tps):
        """PE-transpose src [128,128] bf16 -> dst [128,128] bf16."""
        ps = tps.tile([128, 128], BF16, tag="tp")
        nc.tensor.transpose(ps[:], src_slice, ident[:])
        nc.vector.tensor_copy(dst_slice, ps[:])

    # =====================================================================
    # Phase A: load x, LN1, h1^T; k^T -> AG(Ka),AG(Kb); v -> AG(Va),AG(Vb).
    # Weight DMAs ride the GpSimd queue so they don't serialize behind x.
    # qT for head pairs 0-1 is computed here; 2-7 inside the attention
    # schedule (PE executes in order, so emitting them early would delay
    # the first scores/exp).
    # =====================================================================
    sA2 = ExitStack()   # stays open until the last qT inside phase B
    wq_p = sA2.enter_context(tc.tile_pool(name="wq_p", bufs=2))
    h1Tp = sA2.enter_context(tc.tile_pool(name="h1Tp", bufs=1))
    h1T_all = h1Tp.tile([128, 8 * T], BF16)

    sA = ExitStack()
    lnp = sA.enter_context(tc.tile_pool(name="lnp", bufs=2))
    h1p = sA.enter_context(tc.tile_pool(name="h1p", bufs=1))
    tpsA = sA.enter_context(tc.tile_pool(name="tpsA", bufs=2, space="PSUM"))
    mmpsA = sA.enter_context(tc.tile_pool(name="mmpsA", bufs=2, space="PSUM"))
    stgA = sA.enter_context(tc.tile_pool(name="stgA", bufs=4))

    h1_all = h1p.tile([128, NT * H], BF16)

    for t in range(NT):
        nc.sync.dma_start(x_all[:, t * H:(t + 1) * H], x_d[t * 128:(t + 1) * 128, :])
    wk_sb = wq_p.tile([128, 8 * H], BF16, tag="w3", name="wk_sb")
    for fb in range(8):
        nc.gpsimd.dma_start(wk_sb[:, fb * H:(fb + 1) * H], io["wk"][fb * 128:(fb + 1) * 128, :])
    wq_sb = wq_p.tile([128, 8 * H], BF16, tag="w3", name="wq_sb")
    for fb in range(8):
        nc.gpsimd.dma_start(wq_sb[:, fb * H:(fb + 1) * H], io["wq"][fb * 128:(fb + 1) * 128, :])

    for t in range(NT):
        layer_norm_to(lnp, h1_all[:, t * H:(t + 1) * H], x_all[:, t * H:(t + 1) * H])
        for fb in range(8):
            transpose_128(
                h1T_all[:, fb * T + t * 128: fb * T + (t + 1) * 128],
                h1_all[:, t * H + fb * 128: t * H + (fb + 1) * 128],
                tpsA,
            )

    # k^T feature-major: [128 feats, T] per col-tile, written fp8
    def emit_kt(ct):
        ps = mmpsA.tile([128, T], F32, tag="mm_qk")
        for fb in range(8):
            nc.tensor.matmul(
                ps[:],
                wk_sb[:, fb * H + ct * 128: fb * H + (ct + 1) * 128],
                h1T_all[:, fb * T:(fb + 1) * T],
                start=(fb == 0), stop=(fb == 7),
            )
        ktmp = stgA.tile([128, T], FP8, tag="ktmp")
        nc.scalar.activation(ktmp[:], ps[:], AF.Identity, bias=bk[:, ct:ct + 1])
        dst = cc_k_in_a if ct < 4 else cc_k_in_b
        nc.sync.dma_start(dst[(ct % 4) * 128:(ct % 4 + 1) * 128, :], ktmp[:])

    def emit_qt(ct, pool, tag):
        ps = pool.tile([128, T], F32, tag=tag)
        for fb in range(8):
            nc.tensor.matmul(
                ps[:],
                wq_sb[:, fb * H + ct * 128: fb * H + (ct + 1) * 128],
                h1T_all[:, fb * T:(fb + 1) * T],
                start=(fb == 0), stop=(fb == 7),
            )
        nc.scalar.activation(
            qT_all[:, ct * T:(ct + 1) * T], ps[:], AF.Identity,
            bias=bq[:, ct:ct + 1])

    for ct in range(4):
        emit_kt(ct)
    nc.gpsimd.collective_compute(
        "AllGather", mybir.AluOpType.bypass, replica_groups=GROUPS,
        ins=[cc_k_in_a.opt()], outs=[cc_k_out_a.opt()],
    )
    for ct in range(4, 8):
        emit_kt(ct)
    emit_qt(0, mmpsA, "mm_qk")
    emit_qt(1, mmpsA, "mm_qk")

    # v token-major: [128 tok, H] fp8 (wv reuses the wk slot once kT is done)
    wv_sb = wq_p.tile([128, 8 * H], BF16, tag="w3", name="wv_sb")
    for fb in range(8):
        nc.gpsimd.dma_start(wv_sb[:, fb * H:(fb + 1) * H], io["wv"][fb * 128:(fb + 1) * 128, :])

    def emit_v(t):
        for cc in range(2):
            ps = mmpsA.tile([128, 512], F32, tag="mm_v")
            for fb in range(8):
                nc.tensor.matmul(
                    ps[:],
                    h1T_all[:, fb * T + t * 128: fb * T + (t + 1) * 128],
                    wv_sb[:, fb * H + cc * 512: fb * H + (cc + 1) * 512],
                    start=(fb == 0), stop=False,
                )
            nc.tensor.matmul(ps[:], ones_row[:], bv_bf[:, cc * 512:(cc + 1) * 512],
                             start=False, stop=True)
            vtmp = stgA.tile([128, 512], FP8, tag="vtmp")
            nc.vector.tensor_copy(vtmp[:], ps[:])
            dst = cc_v_in_a if t < 2 else cc_v_in_b
            nc.sync.dma_start(dst[(t % 2) * 128:(t % 2 + 1) * 128, cc * 512:(cc + 1) * 512], vtmp[:])

    emit_v(0); emit_v(1)
    nc.gpsimd.collective_compute(
        "AllGather", mybir.AluOpType.bypass, replica_groups=GROUPS,
        ins=[cc_v_in_a.opt()], outs=[cc_v_out_a.opt()],
    )
    emit_v(2); emit_v(3)
    nc.gpsimd.collective_compute(
        "AllGather", mybir.AluOpType.bypass, replica_groups=GROUPS,
        ins=[cc_v_in_b.opt()], outs=[cc_v_out_b.opt()],
    )
    nc.gpsimd.collective_compute(
        "AllGather", mybir.AluOpType.bypass, replica_groups=GROUPS,
        ins=[cc_k_in_b.opt()], outs=[cc_k_out_b.opt()],
    )

    sA.close()

    # prefetch proj weights during attention (gpsimd queue)
    for hp in range(8):
        nc.gpsimd.dma_start(wproj_sb[:, hp * H:(hp + 1) * H],
                            io["wproj"][hp * 128:(hp + 1) * 128, :])

    # =====================================================================
    # Phase B: attention.
    #   scores^T per key-tile (row-packed head pairs, fp8 q/k), exp on ACT
    #   into a deep fp8 pb ring. ctx token-major: for each (head, token
    #   chunk), psum[tok,65] += pb_chunk^T @ [V|1] accumulated over key
    #   tiles; normalize = per-partition 1/sumexp (tiny DVE ops); PE
    #   transposes restore the feat-major ctx^T layout proj expects.
    # =====================================================================
    sB = ExitStack()
    kpool = sB.enter_context(tc.tile_pool(name="kpool", bufs=4))
    spool = sB.enter_context(tc.tile_pool(name="spool", bufs=2, space="PSUM"))
    cpool = sB.enter_context(tc.tile_pool(name="cpool", bufs=2, space="PSUM"))
    tpsB = sB.enter_context(tc.tile_pool(name="tpsB", bufs=2, space="PSUM"))
    ppool = sB.enter_context(tc.tile_pool(name="ppool", bufs=36))
    rpool = sB.enter_context(tc.tile_pool(name="rpool", bufs=8))
    ctokp = sB.enter_context(tc.tile_pool(name="ctokp", bufs=4))
    vsb = sB.enter_context(tc.tile_pool(name="vsb_p", bufs=1)).tile([128, NKT * NH * VW], FP8, name="vsb")

    def load_kt(hp):
        kt_hp = kpool.tile([128, SKEYS], FP8, tag="kt_hp", name="kt_hp")
        cko, hpo = (cc_k_out_a, hp) if hp < 4 else (cc_k_out_b, hp - 4)
        for g in range(G):
            nc.sync.dma_start(kt_hp[:, g * T:(g + 1) * T],
                              cko[g * 512 + hpo * 128: g * 512 + (hpo + 1) * 128, :])
        return kt_hp

    # ones columns for all key tiles up front (DVE; no data deps)
    for kt in range(NKT):
        blk = vsb[:, kt * NH * VW:(kt + 1) * NH * VW]
        nc.vector.memset(blk.rearrange("p (h x) -> p h x", x=VW)[:, :, DH:VW], 1.0)

    def load_v_half(half_kts, cc_v_out):
        # V from AllGather output straight into the interleaved vsb layout
        for kt in half_kts:
            g, j = kt // 4, (kt % 4) % 2
            blk = vsb[:, kt * NH * VW:(kt + 1) * NH * VW]
            dst = blk.rearrange("p (h x) -> p h x", x=VW)[:, :, 0:DH]
            src = cc_v_out[g * 256 + j * 128: g * 256 + (j + 1) * 128, :]
            nc.sync.dma_start(dst, src.rearrange("p (h d) -> p h d", d=DH))

    # DMA emission order == AllGather completion order (Ka, Va, Vb, Kb)
    kt_tiles = [None] * NHP
    for hp in range(4):
        kt_tiles[hp] = load_kt(hp)
    load_v_half(KT_HALF_A, cc_v_out_a)
    load_v_half(KT_HALF_B, cc_v_out_b)
    for hp in range(4, NHP):
        kt_tiles[hp] = load_kt(hp)

    KT_ORDER = KT_HALF_A + KT_HALF_B
    pb_tiles = {}   # (hp, kt) -> pb tile (live between exp and ctx)
    cps_tiles = {}  # hp -> (cps0, cps1), each [128, 4*VW] f32 (one psum bank)

    def emit_scores(hp):
        kt_hp = kt_tiles[hp]
        for kt in KT_ORDER:
            ps = spool.tile([128, 1024], F32, tag="ps", name="ps")
            nc.tensor.matmul(
                ps[:, 0:512],
                kt_hp[0:64, kt * 128:(kt + 1) * 128],
                qT_all[0:64, hp * T:(hp + 1) * T],
                start=True, stop=True, tile_position=(0, 0),
            )
            nc.tensor.matmul(
                ps[:, 512:1024],
                kt_hp[64:128, kt * 128:(kt + 1) * 128],
                qT_all[64:128, hp * T:(hp + 1) * T],
                start=True, stop=True, tile_position=(64, 0),
            )
            pb = ppool.tile([128, 1024], FP8, tag="pb", name="pb")
            nc.scalar.activation(pb[:], ps[:], AF.Exp)
            pb_tiles[(hp, kt)] = pb

    def emit_ctx(hp, kts, first, last):
        """token-major ctx: psum[tok 128, VW] per (head, token chunk)."""
        if first:
            cps_tiles[hp] = (
                cpool.tile([128, NT * VW], F32, tag="cps", name=f"cps0_{hp}"),
                cpool.tile([128, NT * VW], F32, tag="cps", name=f"cps1_{hp}"),
            )
        cps0, cps1 = cps_tiles[hp]
        for i, kt in enumerate(kts):
            pb = pb_tiles.pop((hp, kt))
            st = first and i == 0
            sp = last and i == len(kts) - 1
            for h, cps in enumerate((cps0, cps1)):
                head = hp * 2 + h
                vv = vsb[:, kt * NH * VW + head * VW: kt * NH * VW + (head + 1) * VW]
                for c in range(NT):
                    nc.tensor.matmul(
                        cps[:, c * VW:(c + 1) * VW],
                        pb[:, h * 512 + c * 128: h * 512 + (c + 1) * 128],
                        vv,
                        start=st, stop=sp,
                    )

    def emit_fin(hp):
        """normalize token-major ctx by 1/sumexp, transpose to ctxT_all."""
        cps0, cps1 = cps_tiles.pop(hp)
        for c in range(NT):
            ctok = ctokp.tile([128, 128], BF16, tag="ctok", name=f"ctok_{hp}_{c}")
            for h, cps in enumerate((cps0, cps1)):
                rs = rpool.tile([128, 1], F32, tag="rs")
                nc.vector.reciprocal(rs[:], cps[:, c * VW + DH: c * VW + DH + 1])
                nc.vector.tensor_scalar_mul(
                    ctok[:, h * DH:(h + 1) * DH],
                    cps[:, c * VW: c * VW + DH], rs[:])
            transpose_128(
                ctxT_all[:, hp * T + c * 128: hp * T + (c + 1) * 128],
                ctok[:], tpsB)

    # schedule: scores two head pairs ahead of ctx; qT 2-7 interleaved just
    # in time; ctx split into AG-half chunks so the PE never long-stalls.
    emit_scores(0)
    emit_qt(2, spool, "ps")
    emit_scores(1)
    emit_qt(3, spool, "ps")
    for hp in range(NHP):
        emit_ctx(hp, KT_HALF_A, first=True, last=False)
        if hp + 2 < NHP:
            emit_scores(hp + 2)
        emit_ctx(hp, KT_HALF_B, first=False, last=True)
        if hp + 4 < NHP:
            emit_qt(hp + 4, spool, "ps")
        emit_fin(hp)

    sB.close()
    sA2.close()

    # =====================================================================
    # Phase C: proj (token-major) + residual -> x2, LN2 -> h2^T
    # =====================================================================
    sCD = ExitStack()
    x2p = sCD.enter_context(tc.tile_pool(name="x2p", bufs=1))
    h2Tp = sCD.enter_context(tc.tile_pool(name="h2Tp", bufs=1))
    h3Tp = sCD.enter_context(tc.tile_pool(name="h3Tp", bufs=1))
    x2_all = x2p.tile([128, NT * H], F32, name="x2_all")
    h2T_all = h2Tp.tile([128, 8 * T], BF16, name="h2T_all")
    h3T_all = h3Tp.tile([128, 32 * T], BF16, name="h3T_all")

    sC = ExitStack()
    lnp2 = sC.enter_context(tc.tile_pool(name="lnp2", bufs=2))
    h2p = sC.enter_context(tc.tile_pool(name="h2p", bufs=1))
    tpsC = sC.enter_context(tc.tile_pool(name="tpsC", bufs=2, space="PSUM"))
    mmpsC = sC.enter_context(tc.tile_pool(name="mmpsC", bufs=2, space="PSUM"))
    stgC = sC.enter_context(tc.tile_pool(name="stgC", bufs=4))

    h2_all = h2p.tile([128, NT * H], BF16)

    for t in range(NT):
        for cc in range(2):
            ps = mmpsC.tile([128, 512], F32, tag="pj")
            for hp in range(8):
                nc.tensor.matmul(
                    ps[:],
                    ctxT_all[:, hp * T + t * 128: hp * T + (t + 1) * 128],
                    wproj_sb[:, hp * H + cc * 512: hp * H + (cc + 1) * 512],
                    start=(hp == 0), stop=False,
                )
            nc.tensor.matmul(ps[:], ones_row[:], bproj_bf[:, cc * 512:(cc + 1) * 512],
                             start=False, stop=True)
            nc.vector.tensor_add(
                x2_all[:, t * H + cc * 512: t * H + (cc + 1) * 512],
                ps[:], x_all[:, t * H + cc * 512: t * H + (cc + 1) * 512])
        layer_norm_to(lnp2, h2_all[:, t * H:(t + 1) * H], x2_all[:, t * H:(t + 1) * H])
        for fb in range(8):
            transpose_128(
                h2T_all[:, fb * T + t * 128: fb * T + (t + 1) * 128],
                h2_all[:, t * H + fb * 128: t * H + (fb + 1) * 128],
                tpsC,
            )

    sC.close()

    # =====================================================================
    # Phase D+E fused: per g-tile: wi matmuls + gelu -> h3T[g], then wo
    # matmuls for output columns 0:512 accumulate into 4 persistent psums.
    # Second pass re-reads h3T for output columns 512:1024.
    # =====================================================================
    sD = ExitStack()
    wip = sD.enter_context(tc.tile_pool(name="wip", bufs=6))
    wop = sD.enter_context(tc.tile_pool(name="wop", bufs=6))
    mmpsD = sD.enter_context(tc.tile_pool(name="mmpsD", bufs=4, space="PSUM"))
    wops = sD.enter_context(tc.tile_pool(name="wops", bufs=1, space="PSUM"))
    outp = sD.enter_context(tc.tile_pool(name="outp", bufs=2))

    NG = FFN // 128  # 32
    psE = [wops.tile([128, 512], F32, tag=f"wo_ps{t}", name=f"wo_ps{t}") for t in range(NT)]
    for g in range(NG):
        wi_g = wip.tile([128, 8, 128], BF16, tag="wi_g", name="wi_g")
        src = io["wi"][g:g + 1, :, :, :].rearrange("o p f c -> (o p) f c")
        nc.gpsimd.dma_start(wi_g[:], src)
        ps = mmpsD.tile([128, T], F32, tag="wi_ps", name="wi_ps")
        for fb in range(8):
            nc.tensor.matmul(
                ps[:], wi_g[:, fb, :], h2T_all[:, fb * T:(fb + 1) * T],
                start=(fb == 0), stop=(fb == 7),
            )
        nc.scalar.activation(h3T_all[:, g * T:(g + 1) * T], ps[:],
                             AF.Gelu_apprx_tanh, bias=bi[:, g:g + 1])
        wo_g = wop.tile([128, 512], BF16, tag="wo_g", name="wo_g")
        nc.gpsimd.dma_start(wo_g[:], io["wo"][g * 128:(g + 1) * 128, 0:512])
        for t in range(NT):
            nc.tensor.matmul(
                psE[t][:],
                h3T_all[:, g * T + t * 128: g * T + (t + 1) * 128],
                wo_g[:],
                start=(g == 0), stop=False,
            )
    for t in range(NT):
        nc.tensor.matmul(psE[t][:], ones_row[:], bo_bf[:, 0:512], start=False, stop=True)
        ot = outp.tile([128, 512], F32, tag="ot", name="ot")
        nc.vector.tensor_add(ot[:], psE[t][:], x2_all[:, t * H: t * H + 512])
        nc.sync.dma_start(out_d[t * 128:(t + 1) * 128, 0:512], ot[:])

    # second pass: output columns 512:1024
    psE2 = [wops.tile([128, 512], F32, tag=f"wo_ps{t}", name=f"wo2_ps{t}") for t in range(NT)]
    for g in range(NG):
        wo_g = wop.tile([128, 512], BF16, tag="wo_g", name="wo_g2")
        nc.gpsimd.dma_start(wo_g[:], io["wo"][g * 128:(g + 1) * 128, 512:1024])
        for t in range(NT):
            nc.tensor.matmul(
                psE2[t][:],
                h3T_all[:, g * T + t * 128: g * T + (t + 1) * 128],
                wo_g[:],
                start=(g == 0), stop=False,
            )
    for t in range(NT):
        nc.tensor.matmul(psE2[t][:], ones_row[:], bo_bf[:, 512:1024], start=False, stop=True)
        ot = outp.tile([128, 512], F32, tag="ot", name="ot2")
        nc.vector.tensor_add(ot[:], psE2[t][:], x2_all[:, t * H + 512: t * H + 1024])
        nc.sync.dma_start(out_d[t * 128:(t + 1) * 128, 512:1024], ot[:])

    sD.close()
    sCD.close()
    s_outer.close()


def _build_program():
    nc = bass.Bass("TRN2", target_bir_lowering=False, debug=False, num_devices=NC)
    io = {}
    io["x"] = nc.dram_tensor("x", [T, H], F32, kind="ExternalInput").ap()
    io["wq"] = nc.dram_tensor("wq", [H, H], BF16, kind="ExternalInput").ap()
    io["wk"] = nc.dram_tensor("wk", [H, H], BF16, kind="ExternalInput").ap()
    io["wv"] = nc.dram_tensor("wv", [H, H], BF16, kind="ExternalInput").ap()
    io["wproj"] = nc.dram_tensor("wproj", [H, H], BF16, kind="ExternalInput").ap()
    io["wi"] = nc.dram_tensor("wi", [FFN // 128, 128, 8, 128], BF16, kind="ExternalInput").ap()
    io["wo"] = nc.dram_tensor("wo", [FFN, H], BF16, kind="ExternalInput").ap()
    io["bq"] = nc.dram_tensor("bq", [128, 8], F32, kind="ExternalInput").ap()
    io["bk"] = nc.dram_tensor("bk", [128, 8], F32, kind="ExternalInput").ap()
    io["bi"] = nc.dram_tensor("bi", [128, 32], F32, kind="ExternalInput").ap()
    io["bv_bf"] = nc.dram_tensor("bv_bf", [1, H], BF16, kind="ExternalInput").ap()
    io["bproj_bf"] = nc.dram_tensor("bproj_bf", [1, H], BF16, kind="ExternalInput").ap()
    io["bo_bf"] = nc.dram_tensor("bo_bf", [1, H], BF16, kind="ExternalInput").ap()
    io["ident"] = nc.dram_tensor("ident", [128, 128], BF16, kind="ExternalInput").ap()
    io["ones_row"] = nc.dram_tensor("ones_row", [1, 128], BF16, kind="ExternalInput").ap()
    io["out"] = nc.dram_tensor("out", [T, H], F32, kind="ExternalOutput").ap()
    with tile.TileContext(nc) as tc:
        _emit(tc, nc, io)
    _split_multiwait(nc)
    return nc


_PROGRAM = None
LAST_RESULTS = None


def kernel(x, ln1_scale, ln1_bias, qkv_w, qkv_b, proj_w, proj_b,
           ln2_scale, ln2_bias, wi_w, wi_b, wo_w, wo_b):
    global _PROGRAM, LAST_RESULTS
    x = np.asarray(x, np.float32)
    ln1_scale = np.asarray(ln1_scale, np.float32); ln1_bias = np.asarray(ln1_bias, np.float32)
    qkv_w = np.asarray(qkv_w, np.float32); qkv_b = np.asarray(qkv_b, np.float32)
    proj_w = np.asarray(proj_w, np.float32); proj_b = np.asarray(proj_b, np.float32)
    ln2_scale = np.asarray(ln2_scale, np.float32); ln2_bias = np.asarray(ln2_bias, np.float32)
    wi_w = np.asarray(wi_w, np.float32); wi_b = np.asarray(wi_b, np.float32)
    wo_w = np.asarray(wo_w, np.float32); wo_b = np.asarray(wo_b, np.float32)

    # fold LN affine params into the next matmul's weights/biases
    qkv_w_eff = ln1_scale[:, None] * qkv_w
    qkv_b_eff = qkv_b + ln1_bias @ qkv_w
    w3 = qkv_w_eff.reshape(H, NH, 3, DH)
    b3 = qkv_b_eff.reshape(NH, 3, DH)
    scale = 1.0 / np.sqrt(np.float32(DH))
    wq = (w3[:, :, 0, :] * scale).reshape(H, H)
    wk = w3[:, :, 1, :].reshape(H, H)
    wv = w3[:, :, 2, :].reshape(H, H)
    bq_v = (b3[:, 0, :] * scale).reshape(H)
    bk_v = b3[:, 1, :].reshape(H)
    bv_v = b3[:, 2, :].reshape(H)
    wi_eff = ln2_scale[:, None] * wi_w
    bi_v = wi_b + ln2_bias @ wi_w

    common = {
        "wq": wq.astype(NPBF16), "wk": wk.astype(NPBF16), "wv": wv.astype(NPBF16),
        "wproj": proj_w.astype(NPBF16),
        "wi": np.ascontiguousarray(
            wi_eff.astype(NPBF16).reshape(8, 128, 32, 128).transpose(2, 1, 0, 3)),
        "wo": wo_w.astype(NPBF16),
        "bq": np.ascontiguousarray(bq_v.reshape(8, 128).T.astype(np.float32)),
        "bk": np.ascontiguousarray(bk_v.reshape(8, 128).T.astype(np.float32)),
        "bi": np.ascontiguousarray(bi_v.reshape(32, 128).T.astype(np.float32)),
        "bv_bf": bv_v.reshape(1, H).astype(NPBF16),
        "bproj_bf": proj_b.reshape(1, H).astype(NPBF16),
        "bo_bf": wo_b.reshape(1, H).astype(NPBF16),
        "ident": np.eye(128, dtype=NPBF16),
        "ones_row": np.ones((1, 128), NPBF16),
    }
    x_flat = x.reshape(B * S, H)
    in_maps = []
    for c in range(NC):
        m = dict(common)
        m["x"] = np.ascontiguousarray(x_flat[c * T:(c + 1) * T, :])
        in_maps.append(m)

    if _PROGRAM is None:
        _PROGRAM = _build_program()
    r = run_bass_kernel_spmd(_PROGRAM, in_maps, list(range(NC)))
    LAST_RESULTS = r
    out = np.concatenate([r.results[c]["out"] for c in range(NC)], axis=0)
    return out.reshape(B, S, H).astype(np.float32)
